# revision 1
# baseline (speedup 1.0000x reference)
"""MinGRU forward on 8 TRN2 NeuronCores.

Math (linear-space reformulation of the reference's log-space Heinsen scan):
    hg = x @ W_hg.T                       # [B,S,2D]
    hidden, gate = split(hg)
    z = sigmoid(gate)
    c = sigmoid(-gate)                    # = 1 - z = exp(-softplus(gate))
    g = max(hidden + 0.5, sigmoid(hidden))  # == where(h>=0, h+0.5, sigmoid(h)) exactly
    u = z * g
    h[t] = c[t] * h[t-1] + u[t]           # convex combination -> bounded, stable
    out = h

The recurrence maps directly onto the DVE `tensor_tensor_scan` instruction
(state = data0*state + data1 along the free dim, fp32 internal state).

Sharding: 8 cores = 4 batches x 2 feature-halves (512 features each).
No cross-core communication: the scan is per-feature independent.
Host pre-transposes x (-> xT [D,S]) and W (-> wT [D, 2*512]) so the kernel
needs no on-chip transposes; matmul uses fp32r (fp32 with 11-bit mantissa,
full-rate on the PE).  Inputs are pre-rounded to fp32r on the host (RNE).
"""

import numpy as np

B, S, D = 4, 4096, 1024
DH = D // 2          # features per core
N_CORES = 8
SC = 512             # tokens per seq chunk (PSUM bank = 512 fp32)
NSC = S // SC        # 8 seq chunks
KC = 128             # contraction chunk
NKC = D // KC        # 8 k chunks
FC = 128             # feature chunk (psum partitions)
NFC = DH // FC       # 4 feature chunks

_CACHE = {}


def _round_fp32r(a: np.ndarray) -> np.ndarray:
    """Round fp32 array to fp32r (11 explicit mantissa bits) with RNE."""
    u = np.ascontiguousarray(a, dtype=np.float32).view(np.uint32)
    r = (u + np.uint32(0x7FF) + ((u >> np.uint32(12)) & np.uint32(1))) & np.uint32(0xFFFFF000)
    return r.view(np.float32)


def _build():
    import concourse.bacc as bacc
    import concourse.tile as tile
    import concourse.mybir as mybir

    f32 = mybir.dt.float32
    f32r = mybir.dt.float32r
    AF = mybir.ActivationFunctionType
    OP = mybir.AluOpType

    nc = bacc.Bacc("TRN2")
    xT = nc.dram_tensor("xT", [D, S], f32r, kind="ExternalInput")
    wT = nc.dram_tensor("wT", [D, 2 * DH], f32r, kind="ExternalInput")
    outT = nc.dram_tensor("outT", [DH, S], f32, kind="ExternalOutput")

    with tile.TileContext(nc) as tc:
        with (
            tc.tile_pool(name="w", bufs=1) as wpool,
            tc.tile_pool(name="x", bufs=2) as xpool,
            tc.tile_pool(name="ew", bufs=3) as epool,
            tc.tile_pool(name="h", bufs=2) as hpool,
            tc.tile_pool(name="ps", bufs=2, space="PSUM") as pspool,
        ):
            wt = wpool.tile([KC, NKC, 2 * DH], f32r)
            nc.sync.dma_start(wt[:], wT.rearrange("(k p) e -> p k e", p=KC))

            hprev = [None] * NFC
            for sc in range(NSC):
                xt = xpool.tile([KC, NKC, SC], f32r)
                nc.sync.dma_start(
                    xt[:], xT[:, sc * SC:(sc + 1) * SC].rearrange("(k p) s -> p k s", p=KC)
                )
                for fc in range(NFC):
                    ph = pspool.tile([FC, SC], f32, tag="ph")
                    pg = pspool.tile([FC, SC], f32, tag="pg")
                    for k in range(NKC):
                        nc.tensor.matmul(
                            ph[:], wt[:, k, fc * FC:(fc + 1) * FC], xt[:, k, :],
                            start=(k == 0), stop=(k == NKC - 1),
                        )
                    for k in range(NKC):
                        nc.tensor.matmul(
                            pg[:], wt[:, k, DH + fc * FC:DH + (fc + 1) * FC], xt[:, k, :],
                            start=(k == 0), stop=(k == NKC - 1),
                        )
                    zt = epool.tile([FC, SC], f32, tag="z")
                    ct = epool.tile([FC, SC], f32, tag="c")
                    st = epool.tile([FC, SC], f32, tag="s")
                    gt = epool.tile([FC, SC], f32, tag="g")
                    ut = epool.tile([FC, SC], f32, tag="u")
                    nc.scalar.activation(zt[:], pg[:], AF.Sigmoid)
                    nc.scalar.activation(ct[:], pg[:], AF.Sigmoid, scale=-1.0)
                    nc.scalar.activation(st[:], ph[:], AF.Sigmoid)
                    # g = (hidden + 0.5) max sigmoid(hidden)
                    nc.vector.scalar_tensor_tensor(
                        gt[:], ph[:], 0.5, st[:], op0=OP.add, op1=OP.max
                    )
                    nc.vector.tensor_mul(ut[:], zt[:], gt[:])
                    ht = hpool.tile([FC, SC], f32, tag=f"h{fc}")
                    init = 0.0 if sc == 0 else hprev[fc][:, SC - 1:SC]
                    nc.vector.tensor_tensor_scan(
                        ht[:], ct[:], ut[:], init, op0=OP.mult, op1=OP.add
                    )
                    hprev[fc] = ht
                    nc.sync.dma_start(
                        outT[fc * FC:(fc + 1) * FC, sc * SC:(sc + 1) * SC], ht[:]
                    )

    nc.compile()
    return nc


def kernel(x: np.ndarray, W_hg: np.ndarray) -> np.ndarray:
    from concourse.bass_utils import run_bass_kernel_spmd

    if "nc" not in _CACHE:
        _CACHE["nc"] = _build()
    nc = _CACHE["nc"]

    x = np.asarray(x, dtype=np.float32)
    W_hg = np.asarray(W_hg, dtype=np.float32)

    xTs = [_round_fp32r(np.ascontiguousarray(x[b].T)) for b in range(B)]
    wTs = []
    for c in range(2):
        w_h = W_hg[c * DH:(c + 1) * DH]          # hidden rows for this half
        w_g = W_hg[D + c * DH:D + (c + 1) * DH]  # gate rows for this half
        wTs.append(_round_fp32r(np.ascontiguousarray(np.concatenate([w_h, w_g], 0).T)))

    in_maps = [{"xT": xTs[core // 2], "wT": wTs[core % 2]} for core in range(N_CORES)]
    res = run_bass_kernel_spmd(nc, in_maps, core_ids=list(range(N_CORES)))

    out = np.empty((B, S, D), dtype=np.float32)
    for core in range(N_CORES):
        b, c = core // 2, core % 2
        out[b, :, c * DH:(c + 1) * DH] = res.results[core]["outT"].T
    return out


# revision 2
# speedup vs baseline: 1.0366x; 1.0366x over previous
"""MinGRU forward on 8 TRN2 NeuronCores.

Math (linear-space reformulation of the reference's log-space Heinsen scan):
    hg = x @ W_hg.T                       # [B,S,2D]
    hidden, gate = split(hg)
    z = sigmoid(gate)
    c = sigmoid(-gate)                    # = 1 - z = exp(-softplus(gate))
    g = max(hidden + 0.5, sigmoid(hidden))  # == where(h>=0, h+0.5, sigmoid(h)) exactly
    u = z * g
    h[t] = c[t] * h[t-1] + u[t]           # convex combination -> bounded, stable
    out = h

The recurrence maps directly onto the DVE `tensor_tensor_scan` instruction
(state = data0*state + data1 along the free dim, fp32 internal state).

Sharding: 8 cores = 4 batches x 2 feature-halves (512 features each).
No cross-core communication: the scan is per-feature independent.
Host pre-transposes x (-> xT [D,S]) and W (-> wT [D, 2*512]) so the kernel
needs no on-chip transposes; matmul uses fp32r (fp32 with 11-bit mantissa,
full-rate on the PE).  Inputs are pre-rounded to fp32r on the host (RNE).
"""

import numpy as np

B, S, D = 4, 4096, 1024
DH = D // 2          # features per core
N_CORES = 8
SC = 512             # tokens per seq chunk (PSUM bank = 512 fp32)
NSC = S // SC        # 8 seq chunks
KC = 128             # contraction chunk
NKC = D // KC        # 8 k chunks
FC = 128             # feature chunk (psum partitions)
NFC = DH // FC       # 4 feature chunks

_CACHE = {}


def _round_fp32r(a: np.ndarray) -> np.ndarray:
    """Round fp32 array to fp32r (11 explicit mantissa bits) with RNE."""
    u = np.ascontiguousarray(a, dtype=np.float32).view(np.uint32)
    r = (u + np.uint32(0x7FF) + ((u >> np.uint32(12)) & np.uint32(1))) & np.uint32(0xFFFFF000)
    return r.view(np.float32)


def _build():
    import concourse.bacc as bacc
    import concourse.tile as tile
    import concourse.mybir as mybir

    f32 = mybir.dt.float32
    f32r = mybir.dt.float32r
    AF = mybir.ActivationFunctionType
    OP = mybir.AluOpType

    nc = bacc.Bacc("TRN2")
    xT = nc.dram_tensor("xT", [D, S], f32r, kind="ExternalInput")
    wT = nc.dram_tensor("wT", [D, 2 * DH], f32r, kind="ExternalInput")
    outT = nc.dram_tensor("outT", [DH, S], f32, kind="ExternalOutput")

    with tile.TileContext(nc) as tc:
        with (
            tc.tile_pool(name="w", bufs=1) as wpool,
            tc.tile_pool(name="x", bufs=2) as xpool,
            tc.tile_pool(name="ew", bufs=3) as epool,
            tc.tile_pool(name="h", bufs=2) as hpool,
            tc.tile_pool(name="ps", bufs=2, space="PSUM") as pspool,
        ):
            wt = wpool.tile([KC, NKC, 2 * DH], f32r)
            wT_r = wT.rearrange("(k p) e -> p k e", p=KC)
            for k in range(NKC):
                nc.sync.dma_start(wt[:, k, :], wT_r[:, k, :])

            hprev = [None] * NFC
            for sc in range(NSC):
                xt = xpool.tile([KC, NKC, SC], f32r)
                xT_r = xT[:, sc * SC:(sc + 1) * SC].rearrange("(k p) s -> p k s", p=KC)
                for k in range(NKC):
                    nc.sync.dma_start(xt[:, k, :], xT_r[:, k, :])
                for fc in range(NFC):
                    ph = pspool.tile([FC, SC], f32, tag="ph")
                    pg = pspool.tile([FC, SC], f32, tag="pg")
                    for k in range(NKC):
                        nc.tensor.matmul(
                            ph[:], wt[:, k, fc * FC:(fc + 1) * FC], xt[:, k, :],
                            start=(k == 0), stop=(k == NKC - 1),
                        )
                    for k in range(NKC):
                        nc.tensor.matmul(
                            pg[:], wt[:, k, DH + fc * FC:DH + (fc + 1) * FC], xt[:, k, :],
                            start=(k == 0), stop=(k == NKC - 1),
                        )
                    zt = epool.tile([FC, SC], f32, tag="z")
                    ct = epool.tile([FC, SC], f32, tag="c")
                    st = epool.tile([FC, SC], f32, tag="s")
                    gt = epool.tile([FC, SC], f32, tag="g")
                    ut = epool.tile([FC, SC], f32, tag="u")
                    nc.scalar.activation(zt[:], pg[:], AF.Sigmoid)
                    nc.scalar.activation(ct[:], pg[:], AF.Sigmoid, scale=-1.0)
                    nc.scalar.activation(st[:], ph[:], AF.Sigmoid)
                    # g = (hidden + 0.5) max sigmoid(hidden)
                    nc.vector.scalar_tensor_tensor(
                        gt[:], ph[:], 0.5, st[:], op0=OP.add, op1=OP.max
                    )
                    nc.vector.tensor_mul(ut[:], zt[:], gt[:])
                    ht = hpool.tile([FC, SC], f32, tag=f"h{fc}")
                    init = 0.0 if sc == 0 else hprev[fc][:, SC - 1:SC]
                    nc.vector.tensor_tensor_scan(
                        ht[:], ct[:], ut[:], init, op0=OP.mult, op1=OP.add
                    )
                    hprev[fc] = ht
                    nc.sync.dma_start(
                        outT[fc * FC:(fc + 1) * FC, sc * SC:(sc + 1) * SC], ht[:]
                    )

    nc.compile()
    return nc


def kernel(x: np.ndarray, W_hg: np.ndarray) -> np.ndarray:
    from concourse.bass_utils import run_bass_kernel_spmd

    if "nc" not in _CACHE:
        _CACHE["nc"] = _build()
    nc = _CACHE["nc"]

    x = np.asarray(x, dtype=np.float32)
    W_hg = np.asarray(W_hg, dtype=np.float32)

    xTs = [_round_fp32r(np.ascontiguousarray(x[b].T)) for b in range(B)]
    wTs = []
    for c in range(2):
        w_h = W_hg[c * DH:(c + 1) * DH]          # hidden rows for this half
        w_g = W_hg[D + c * DH:D + (c + 1) * DH]  # gate rows for this half
        wTs.append(_round_fp32r(np.ascontiguousarray(np.concatenate([w_h, w_g], 0).T)))

    in_maps = [{"xT": xTs[core // 2], "wT": wTs[core % 2]} for core in range(N_CORES)]
    res = run_bass_kernel_spmd(nc, in_maps, core_ids=list(range(N_CORES)))

    out = np.empty((B, S, D), dtype=np.float32)
    for core in range(N_CORES):
        b, c = core // 2, core % 2
        out[b, :, c * DH:(c + 1) * DH] = res.results[core]["outT"].T
    return out
